# revision 79
# baseline (speedup 1.0000x reference)
"""Trainium2 Bass kernel for nn_AttentionModule (B=4, C=512, N=4096, CQK=64).

Sharding: 8 cores = (batch b, query-half h). Each core receives x[b] with
columns rotated so that its 2048-query slab is always columns 0:2048 —
attention output for query i depends on the full key set but is invariant
to key permutation, so rotation keeps the program identical across cores.

Numerics (max-rel-err budget 2e-2; this lands ~9e-3): the worst output
errors occur at peaked softmax rows where logit noise directly modulates
the dominant weight, so the q/k path runs in bf16 — projection from bf16
x (host-cast) with bf16 weights, logits as row-packed bf16 matmuls (even
j-tile on PE rows 0:64, odd on 64:128, k/q duplicated across halves via
SBUF DMA). Everything else runs as fp8e4m3 DoubleRow matmuls (0.5
cycles/row): v projection from x/8 (host-cast fp8) with 8x-scaled
weights, E = exp(logits - 6) written by ACT straight into an fp8 arena
(logit max ~11 -> E max ~143 < 448), softmax denominator as a ones-matmul
over arena pairs (f32 PSUM accumulation), and AV over 16 DoubleRow pairs
per c-tile. Out stage: out = av * recip (DVE) + x_slab (gpsimd).

PSUM: 4-bank + 2-bank logit groups (double-buffered against each other,
amortizing the ACT per-op bubble) + a 2-slot [128,512] ring for
kq/v/denominator/AV accumulators = exactly 8 banks.
"""

import sys

if "/opt/trn_rl_repo" not in sys.path:
    sys.path.insert(0, "/opt/trn_rl_repo")

from contextlib import ExitStack

import ml_dtypes
import numpy as np

import concourse.tile as tile
from concourse import bacc, mybir
from concourse.bass_utils import run_bass_kernel_spmd

B, C, N = 4, 512, 4096
CQK = C // 8
NCORES = 8
SLAB = N // 2            # queries per core
CHUNK = 512              # matmul moving free dim
NCHUNK = N // CHUNK      # 8 column chunks of x
NKT = C // 128           # 4 contraction tiles over input channels
NJT = N // 128           # 32 key tiles
NBLK = SLAB // CHUNK     # 4 query blocks per core
EXP_BIAS = -6.0          # exp range shift: logits max ~11 -> E max ~143
DITHER = 1.0625          # grid offset between the two k/q fp8 quantizations

# logits/exp group sizes (in j-tiles) per block; 4-tile groups use the
# 4-bank psum pool, 2-tile groups the 2-bank pool, alternating so they
# double-buffer against each other. sum == NJT. First group small so the
# ACT engine starts on block 0 as early as possible.
GROUPS = [2, 4, 2, 4, 2, 4, 2, 4, 2, 4, 2]
GSTART = [0]
for _g in GROUPS:
    GSTART.append(GSTART[-1] + _g)
# chunk whose k-projection a block-0 group needs last
READY_AT = [(GSTART[g + 1] - 1) // 4 for g in range(len(GROUPS))]

F32 = mybir.dt.float32
F8 = mybir.dt.float8e4
BF16 = mybir.dt.bfloat16
I32 = mybir.dt.int32

# Schraudolph exp: exp(l + EXP_BIAS) ~= bitcast_f32(int32(l*SCH_A + SCH_B)).
# Groups listed here run on DVE (affine+int cast) and gpsimd (bitcast->fp8)
# instead of ACT; the ~2% sawtooth washes out in the softmax (verified to
# not move max rel err at all). Targeting the LAST groups of each later
# block also lets arena3 finish off ACT's critical path, shrinking the tail.
import math as _math
SCH_A = float(2 ** 23 / _math.log(2.0))
SCH_B = float(127 * 2 ** 23 - 366393 + SCH_A * EXP_BIAS)
OFFLOAD = set()
FP8NP = ml_dtypes.float8_e4m3fn
BF16NP = ml_dtypes.bfloat16
DR = mybir.MatmulPerfMode.DoubleRow

_compiled = None


def _build():
    nc = bacc.Bacc("TRN2", debug=False, num_devices=NCORES)

    xbf_d = nc.dram_tensor("xbf", [C, N], BF16, kind="ExternalInput").ap()
    x8_d = nc.dram_tensor("x8", [C, N], F8, kind="ExternalInput").ap()
    xs_d = nc.dram_tensor("xslab", [C, SLAB], F32, kind="ExternalInput").ap()
    wkq_d = nc.dram_tensor("wkq", [128, NKT * 128], BF16,
                           kind="ExternalInput").ap()
    wv_d = nc.dram_tensor("wv", [128, NKT * CHUNK], F8,
                          kind="ExternalInput").ap()
    bkq_d = nc.dram_tensor("bkq", [128, 1], F32, kind="ExternalInput").ap()
    ones_d = nc.dram_tensor("ones", [128, 256], F8, kind="ExternalInput").ap()
    out_d = nc.dram_tensor("out", [C, SLAB], F32, kind="ExternalOutput").ap()

    Exp = mybir.ActivationFunctionType.Exp

    with tile.TileContext(nc) as tc, ExitStack() as ctx:
        consts = ctx.enter_context(tc.tile_pool(name="consts", bufs=1))
        kqv = ctx.enter_context(tc.tile_pool(name="kqv", bufs=1))
        kqfpool = ctx.enter_context(tc.tile_pool(name="kqf", bufs=2))
        spool = ctx.enter_context(tc.tile_pool(name="sch", bufs=2))
        apool = ctx.enter_context(tc.tile_pool(name="arena", bufs=4))
        rpool = ctx.enter_context(tc.tile_pool(name="recip", bufs=2))
        xrpool = ctx.enter_context(tc.tile_pool(name="xr", bufs=2))
        tpool = ctx.enter_context(tc.tile_pool(name="t", bufs=3))
        opool = ctx.enter_context(tc.tile_pool(name="o", bufs=2))
        big_ps = ctx.enter_context(tc.tile_pool(name="bigps", bufs=1,
                                                space="PSUM"))
        med_ps = ctx.enter_context(tc.tile_pool(name="medps", bufs=1,
                                                space="PSUM"))
        av_ps = ctx.enter_context(tc.tile_pool(name="avps", bufs=2,
                                               space="PSUM"))

        # --- constants ---
        wkq = consts.tile([128, NKT * 128], BF16, tag="wkq")
        wv = consts.tile([128, NKT * CHUNK], F8, tag="wv")
        bkq = consts.tile([128, 1], F32, tag="bkq")
        ones = consts.tile([128, 256], F8, tag="ones")
        ebias = consts.tile([128, 1], F32, tag="ebias")
        nc.vector.memset(ebias[:], EXP_BIAS)

        wv3 = wv[:].rearrange("p (t o) -> p t o", t=NKT)
        ones3 = ones[:].rearrange("p (two o) -> p two o", two=2)

        # k/q stored as two dither-offset fp8 quantizations: the DoubleRow
        # pair computes fp8_a(k/2).fp8_a(q) + fp8_b(k*D/2).fp8_b(q/D) = k.q
        # with the two grids offset by D so cast errors partially average out
        k2 = kqv.tile([CQK, 2 * N], F8, tag="k2")
        q2 = kqv.tile([CQK, 2 * SLAB], F8, tag="q2")
        k23 = k2[:].rearrange("p (two n) -> p two n", two=2)
        q23 = q2[:].rearrange("p (two n) -> p two n", two=2)
        vt = kqv.tile([128, NJT * C], F8, tag="vt")
        vt3 = vt[:].rearrange("p (j c) -> p j c", j=NJT)

        def dr(out, lhsT, rhs, start, stop):
            nc.tensor.matmul(out, lhsT, rhs, start=start, stop=stop,
                             perf_mode=DR)

        arenas = {}

        def emit_group(blk, g):
            """Logits + exp for j-tiles GSTART[g]:GSTART[g+1] of block blk."""
            if blk not in arenas:
                arenas[blk] = apool.tile([128, NJT * CHUNK], F8, tag="arena",
                                         name=f"arena{blk}")
            jt0, njt = GSTART[g], GROUPS[g]
            pool = big_ps if njt == 4 else med_ps
            lp = pool.tile([128, njt * CHUNK], F32,
                           tag="big" if njt == 4 else "med",
                           name=f"l{blk}_{g}")
            icols = slice(blk * CHUNK, (blk + 1) * CHUNK)
            for j in range(njt):
                jt = jt0 + j
                dr(lp[:, j * CHUNK:(j + 1) * CHUNK],
                   k23[:, :, jt * 128:(jt + 1) * 128], q23[:, :, icols],
                   True, True)
            dst = arenas[blk][:, jt0 * CHUNK:(jt0 + njt) * CHUNK]
            if (blk, g) in OFFLOAD:
                sb_i = spool.tile([128, 4 * CHUNK], I32, tag="sch",
                                  name=f"sch{blk}_{g}")
                w = njt * CHUNK
                nc.vector.tensor_scalar(sb_i[:, 0:w], lp[:], SCH_A, SCH_B,
                                        MUL, ADD)
                nc.gpsimd.tensor_copy(dst, sb_i[:, 0:w].bitcast(F32))
            else:
                nc.scalar.activation(dst, lp[:], Exp, bias=ebias[:],
                                     scale=1.0)

        # --- x loads. Every DMA costs ~0.6us of serialized HWDGE descriptor
        # time, so batch into few transfers; xbf chunk 0 goes absolutely
        # first so the k/q pipeline (and hence ACT) starts early.
        xbf_a = kqv.tile([128, NKT * N], BF16, tag="xbfa")
        x8_a = kqv.tile([128, NKT * N], F8, tag="x8a")
        xbf3 = xbf_a[:].rearrange("p (t n) -> p t n", t=NKT)
        x83a = x8_a[:].rearrange("p (t n) -> p t n", t=NKT)

        def load_x(dst3, src_d, eng, c0, c1):
            eng.dma_start(
                dst3[:, :, c0:c1],
                src_d[:, c0:c1].rearrange("(t p) n -> p t n", t=NKT))

        # single ring, strict order: weights first (tiny transfers, and kq0
        # blocks on their completion semaphores), then the k/q-critical xbf
        # chunks with the v-path x8 halves interleaved so vt casts can start
        # well before AV0 needs them
        # single-chunk xbf DMAs: each chunk's completion semaphore fires as
        # soon as its own ~1.6us transfer lands (batched loads delayed a
        # chunk's availability to the end of its batch); descriptors
        # pipeline under the transfers
        nc.sync.dma_start(wkq[:], wkq_d[:])
        nc.sync.dma_start(bkq[:], bkq_d[:])
        for ch in range(5):
            load_x(xbf3, xbf_d, nc.sync, ch * CHUNK, (ch + 1) * CHUNK)
        nc.sync.dma_start(wv[:], wv_d[:])
        load_x(x83a, x8_d, nc.sync, 0, N // 2)
        for ch in range(5, NCHUNK):
            load_x(xbf3, xbf_d, nc.sync, ch * CHUNK, (ch + 1) * CHUNK)
        load_x(x83a, x8_d, nc.sync, N // 2, N)
        nc.sync.dma_start(ones[:], ones_d[:])

        # PE warmup: ~10 throwaway matmuls on a memset tile ramp the tensor
        # engine to full p-state before the first real projection arrives
        warm = consts.tile([128, CHUNK], BF16, tag="warm")
        nc.gpsimd.memset(warm[:], 0.0)
        wu_ps = av_ps.tile([128, CHUNK], F32, tag="av", name="warmup")
        for w in range(9):
            nc.tensor.matmul(wu_ps[:], warm[:, 0:128], warm[:],
                             start=(w == 0), stop=(w == 8))

        # --- phase A1: k/q projections + block-0 logits.
        # DVE stages k|q as one f32 op per chunk; the four dithered fp8
        # casts run on gpsimd (SBUF->SBUF), keeping DVE free for vt later.
        ADD, MUL = mybir.AluOpType.add, mybir.AluOpType.mult
        for ch in range(NCHUNK):
            cols = slice(ch * CHUNK, (ch + 1) * CHUNK)
            kq_ps = av_ps.tile([128, CHUNK], F32, tag="av", name=f"kq{ch}")
            nrow = 128 if ch < NBLK else CQK
            for t in range(NKT):
                nc.tensor.matmul(kq_ps[0:nrow, :],
                                 wkq[:, t * 128:t * 128 + nrow],
                                 xbf3[:, t, cols],
                                 start=(t == 0), stop=(t == NKT - 1))
            c0, c1 = ch * CHUNK, (ch + 1) * CHUNK
            # q staged first (its Pool-cast leg is the longer path to the
            # first logits group), then the k dither casts straight from
            # PSUM on DVE; the q dither casts on gpsimd use a partition
            # remap 64:128 -> 0:64, which the vector engines permit
            if ch < NBLK:
                kqf = kqfpool.tile([CQK, CHUNK], F32, tag="kqf",
                                   name=f"kqf{ch}")
                nc.vector.tensor_scalar_add(kqf[:], kq_ps[CQK:128, :],
                                            bkq[CQK:128])
                nc.gpsimd.tensor_copy(q2[:, c0:c1], kqf[:])
                nc.gpsimd.tensor_scalar_mul(q2[:, SLAB + c0:SLAB + c1],
                                            kqf[:], 1.0 / DITHER)
            nc.vector.tensor_scalar(k2[:, c0:c1], kq_ps[0:CQK, :],
                                    bkq[0:CQK], 0.5, ADD, MUL)
            nc.vector.tensor_scalar(k2[:, N + c0:N + c1], kq_ps[0:CQK, :],
                                    bkq[0:CQK], 0.5 * DITHER, ADD, MUL)
            for g in range(len(GROUPS)):
                if READY_AT[g] == ch:
                    emit_group(0, g)

        # early block-1 logits to keep ACT fed across the phase boundary
        emit_group(1, 0)
        emit_group(1, 1)

        # --- phase A2: v projections (fp8 DoubleRow); vt casts queue on DVE
        # strictly after all k/q staging ops
        # vt bias folded into the out stage (sum_j E*(v+bv) recip = av recip
        # + bv since denom*recip == 1), so the PSUM->fp8 cast is a pure copy
        # and ACT's idle gaps can absorb a quarter of them alongside DVE
        Copy = mybir.ActivationFunctionType.Copy
        for jt in range(NJT):
            v_ps = av_ps.tile([128, CHUNK], F32, tag="av", name=f"v{jt}")
            for s in range(2):
                dr(v_ps[:],
                   x83a[:, 2 * s:2 * s + 2, jt * 128:(jt + 1) * 128],
                   wv3[:, 2 * s:2 * s + 2, :], s == 0, s == 1)
            nc.vector.tensor_copy(vt[:, jt * C:(jt + 1) * C], v_ps[:])

        # --- phase B: flat schedule. AV work for block b is emitted one
        # section later, interleaved BETWEEN the logits-group emissions of
        # block b+2, so PE's in-order stream never parks on vt/arena-gated
        # AV matmuls while ACT still has logits to chew on.
        xrs, rcs, oos = {}, {}, {}

        def load_xr(blk):
            icols = slice(blk * CHUNK, (blk + 1) * CHUNK)
            xr = xrpool.tile([128, NKT * CHUNK], F32, tag="xr",
                             name=f"xr{blk}")
            nc.sync.dma_start(
                xr[:].rearrange("p (c i) -> p c i", c=NKT),
                xs_d[:, icols].rearrange("(c p) i -> p c i", c=NKT))
            xrs[blk] = xr

        def emit_denom(blk):
            arena3 = arenas[blk][:].rearrange("p (j i) -> p j i", j=NJT)
            s_ps = av_ps.tile([128, CHUNK], F32, tag="av", name=f"s{blk}")
            for t in range(NJT // 2):
                dr(s_ps[:], ones3[:],
                   arena3[:, 2 * t:2 * t + 2, :], t == 0, t == NJT // 2 - 1)
            rc = rpool.tile([128, CHUNK], F32, tag="recip", name=f"rc{blk}")
            nc.vector.reciprocal(rc[:], s_ps[:])
            rcs[blk] = rc

        def emit_av_c(blk, c):
            icols = slice(blk * CHUNK, (blk + 1) * CHUNK)
            arena3 = arenas[blk][:].rearrange("p (j i) -> p j i", j=NJT)
            if blk not in oos:
                oos[blk] = opool.tile([128, NKT * CHUNK], F32, tag="o",
                                      name=f"oo{blk}")
            oo = oos[blk]
            av = av_ps.tile([128, CHUNK], F32, tag="av", name=f"av{blk}_{c}")
            for t in range(NJT // 2):
                dr(av[:], vt3[:, 2 * t:2 * t + 2, c * 128:(c + 1) * 128],
                   arena3[:, 2 * t:2 * t + 2, :], t == 0, t == NJT // 2 - 1)
            tm = tpool.tile([128, CHUNK], F32, tag="t", name=f"tm{blk}_{c}")
            nc.vector.tensor_mul(tm[:], av[:], rcs[blk][:])
            # last block: keep the add on DVE back-to-back with the mult
            # (saves the slower Pool op + a cross-engine hop in the tail)
            eng = nc.vector if blk == NBLK - 1 else nc.gpsimd
            eng.tensor_add(oo[:, c * CHUNK:(c + 1) * CHUNK], tm[:],
                           xrs[blk][:, c * CHUNK:(c + 1) * CHUNK])
            if blk == NBLK - 1:
                nc.sync.dma_start(out_d[c * 128:(c + 1) * 128, icols],
                                  oo[:, c * CHUNK:(c + 1) * CHUNK])
            elif c == NKT - 1:
                nc.sync.dma_start(
                    out_d[:, icols].rearrange("(c p) i -> p c i", c=NKT),
                    oo[:].rearrange("p (c i) -> p c i", c=NKT))

        # section 0: block-1 logits, block-0 denominator
        load_xr(0)
        emit_group(1, 2)
        emit_group(1, 3)
        emit_denom(0)
        for g in range(4, len(GROUPS)):
            emit_group(1, g)
        emit_group(2, 0)
        emit_group(2, 1)
        # section 1: block-2 logits with AV(0) interleaved
        load_xr(1)
        emit_group(2, 2)
        emit_group(2, 3)
        emit_av_c(0, 0)
        emit_av_c(0, 1)
        emit_group(2, 4)
        emit_group(2, 5)
        emit_av_c(0, 2)
        emit_av_c(0, 3)
        for g in range(6, len(GROUPS)):
            emit_group(2, g)
        emit_group(3, 0)
        emit_group(3, 1)
        emit_denom(1)
        # section 2: block-3 logits with AV(1) AND AV(2) interleaved, so the
        # tail section is only denom(3)+AV(3) after the last exp
        load_xr(2)
        emit_group(3, 2)
        emit_group(3, 3)
        emit_av_c(1, 0)
        emit_av_c(1, 1)
        emit_group(3, 4)
        emit_group(3, 5)
        emit_av_c(1, 2)
        emit_av_c(1, 3)
        emit_group(3, 6)
        emit_group(3, 7)
        emit_denom(2)
        emit_av_c(2, 0)
        emit_group(3, 8)
        emit_av_c(2, 1)
        emit_group(3, 9)
        emit_av_c(2, 2)
        emit_group(3, 10)
        emit_av_c(2, 3)
        # section 3: tail
        load_xr(3)
        emit_denom(3)
        for c in range(NKT):
            emit_av_c(3, c)

    nc.compile()
    return nc


def _get_compiled():
    global _compiled
    if _compiled is None:
        _compiled = _build()
    return _compiled


def kernel(x, Wq, bq, Wk, bk, Wv, bv, gamma, **run_kwargs):
    x = np.asarray(x, dtype=np.float32)
    Wq = np.asarray(Wq, dtype=np.float32)
    bq = np.asarray(bq, dtype=np.float32)
    Wk = np.asarray(Wk, dtype=np.float32)
    bk = np.asarray(bk, dtype=np.float32)
    Wv = np.asarray(Wv, dtype=np.float32)
    bv = np.asarray(bv, dtype=np.float32)
    g = float(np.asarray(gamma).reshape(-1)[0])

    # q/k path bf16: [p, t, o] = W[o, t*128+p] with k rows 0:64, q 64:128
    wkq_full = np.concatenate([Wk, Wq], axis=0)  # [128, C]
    wkq_h = np.ascontiguousarray(
        wkq_full.T.reshape(NKT, 128, 128).transpose(1, 0, 2)
        .reshape(128, NKT * 128)).astype(BF16NP)
    # v path fp8: weights 8x so fp8 keeps full relative precision against
    # the x/8 activations; gamma folded in
    wv_h = np.ascontiguousarray(
        (8.0 * g * Wv).T.reshape(NKT, 128, C).transpose(1, 0, 2)
        .reshape(128, NKT * C)).astype(FP8NP)
    shared = {
        "wkq": wkq_h,
        "wv": wv_h,
        "bkq": np.ascontiguousarray(
            np.concatenate([bk, bq]).reshape(128, 1)),
        "ones": np.ones((128, 256), dtype=FP8NP),
    }
    xbf = [x[b].astype(BF16NP) for b in range(B)]
    x8 = [(x[b] * 0.125).astype(FP8NP) for b in range(B)]
    in_maps = []
    for core in range(NCORES):
        b, h = divmod(core, 2)
        xbfb, x8b = xbf[b], x8[b]
        if h:
            xbfb = np.concatenate([xbfb[:, SLAB:], xbfb[:, :SLAB]], axis=1)
            x8b = np.concatenate([x8b[:, SLAB:], x8b[:, :SLAB]], axis=1)
        in_maps.append({
            "xbf": np.ascontiguousarray(xbfb),
            "x8": np.ascontiguousarray(x8b),
            # residual slab with the v-bias folded in host-side:
            # out = av*recip + (x + gamma*bv) since denom*recip == 1
            "xslab": np.ascontiguousarray(
                x[b][:, h * SLAB:(h + 1) * SLAB] +
                (g * bv).astype(np.float32)[:, None]),
            **shared,
        })

    nc = _get_compiled()
    res = run_bass_kernel_spmd(nc, in_maps, core_ids=list(range(NCORES)),
                               **run_kwargs)

    out = np.empty((B, C, N), dtype=np.float32)
    for core in range(NCORES):
        b, h = divmod(core, 2)
        out[b][:, h * SLAB:(h + 1) * SLAB] = res.results[core]["out"]
    if run_kwargs:
        kernel.last_results = res
    return out
